# revision 45
# baseline (speedup 1.0000x reference)
"""Trainium2 Bass kernel for nn_MicroAdder (tiny dense transformer).

v6 — single K=128 mm2 per block (4 normalized rows for t<30, classic l-rows
for t 30..33), f16 chain, interleaved emission, queue-balanced DMA.

Decomposition: per-element quantities are affine in the basis
[cos(ang), sin(ang), 1] per t. mm1 (per 128-row block) produces 8 linear
forms (q0, q1, a, e0, e1, y0, y1, r); a short f16 chain produces z0, z1,
inv3; mm2 expands to (T, V) logits:

  t < 30:  rows (z0*inv3, z1*inv3, y0*inv3, y1*inv3) x constant v-tables
  t >= 30: rows (l0, l1) = ((y0 + H00 z0 + H10 z1)*inv3,
                            (y1 + H01 z0 + H11 z1)*inv3) x (E0v, E1v)

4*30 + 2*4 = 128 rows -> ONE [128, 476] matmul per block; the tiny l-rows
for 4 t's cost a handful of [P, 64]-wide DVE ops per supergroup.

Sharding: pure data parallel over the batch dim across 8 NeuronCores.
"""

import math
import sys

import numpy as np

for _p in ("/opt/trn_rl_repo", "/root/.axon_site/_ro/trn_rl_repo"):
    if _p not in sys.path:
        sys.path.append(_p)

import ml_dtypes  # noqa: E402

import concourse.bacc as bacc  # noqa: E402
import concourse.bass as bass  # noqa: E402
import concourse.tile as tile  # noqa: E402
from concourse import mybir  # noqa: E402
from concourse.bass_utils import run_bass_kernel_spmd  # noqa: E402

# ---------------------------------------------------------------- problem dims
B, T, V = 65536, 34, 14
D, EPS, MAX_DIGITS = 5, 1e-5, 10
NCORES = 8
BC = B // NCORES            # rows per core = 8192
P = 128                     # partitions
NPER = BC // P              # rows per partition = 64
NBLK = NPER                 # blocks per core = 64
SGB = 16                    # blocks per supergroup
NSG = NBLK // SGB           # 4 supergroups
KB = 128                    # padded basis rows (69 used)
NG = 8                      # matmul1 groups
N1 = NG * T                 # 272
N2 = T * V                  # 476
TM = 30                     # t's in normalized-row (4-row) form
TL = T - TM                 # t's in l-row form = 4
NPRM = 8
FW = T * SGB                # 544 columns per supergroup

F32 = mybir.dt.float32
F16 = mybir.dt.float16
U8 = mybir.dt.uint8
AF = mybir.ActivationFunctionType
ALU = mybir.AluOpType
F16NP = np.float16

# group order in matmul1 output columns (g*T..g*T+T); Q first so the non-Q
# tail [2T:8T] is one contiguous copy
G_Q0, G_Q1, G_A, G_E0, G_E1, G_Y0, G_Y1, G_R = range(8)
NQ_A, NQ_E0, NQ_E1, NQ_Y0, NQ_Y1, NQ_R = range(6)
NQW = 6 * T                 # nonq tile width per block = 204

# lint column map (128 per block):
#   [q*TM : (q+1)*TM]  q in {z~0, z~1, y~0, y~1}   (t < TM)
#   [4*TM + c*TL : ...] c in {l0, l1}               (t in [TM, T))
LOFF = 4 * TM               # 120

# PRM slots (activation bias/scale vectors; chain scalars are baked)
(P_SSC, P_SB, P_CSC, P_CB, P_ZERO) = range(5)


# ---------------------------------------------------------------- host tables
def host_tables(tok_A, tok_start, tok_stride, sp_amp, sp_phase, sp_slope, sp_offset,
                norm_w, q_w, q_phase, out_A, out_B, fc1_w, fc2_w, head_w):
    f = np.float64
    A = f(tok_A)
    t = np.arange(T, dtype=f)
    th = 2.0 * np.pi * t / MAX_DIGITS + f(sp_phase)
    pos = np.stack([f(sp_amp) * np.cos(th), f(sp_amp) * np.sin(th),
                    f(sp_slope) * t + f(sp_offset)], axis=-1)
    k = pos @ np.asarray(q_w, f).T
    c0, s0 = np.cos(f(q_phase[0])), np.sin(f(q_phase[0]))
    q = k.copy()
    q[:, 0] = c0 * k[:, 0] - s0 * k[:, 1]
    q[:, 1] = s0 * k[:, 0] + c0 * k[:, 1]
    scores = (q @ k.T) / np.sqrt(f(5.0))
    sm = np.where(np.tril(np.ones((T, T), bool)), scores, -np.inf)
    sm = sm - sm.max(-1, keepdims=True)
    e = np.exp(sm)
    attn = e / e.sum(-1, keepdims=True)

    nw = np.asarray(norm_w, f)
    oA = np.asarray(out_A, f)[:, 0]
    oB = np.asarray(out_B, f)[0]
    S_t = A * A + (pos ** 2).sum(-1)
    rms1 = np.sqrt(S_t / D + EPS)

    M0 = attn * (A * nw[0] * oA[0] / rms1)[None, :]
    M1 = attn * (A * nw[1] * oA[1] / rms1)[None, :]
    c_t = attn @ ((pos * (nw[2:] * oA[2:])[None, :]).sum(-1) / rms1)

    g0 = np.asarray(fc2_w, f)[:, 0]
    g1 = np.asarray(fc2_w, f)[:, 1]
    hv0 = nw * np.asarray(head_w, f)[0]
    hv1 = nw * np.asarray(head_w, f)[1]
    projs = {
        G_Q0: nw * np.asarray(fc1_w, f)[0],
        G_Q1: nw * np.asarray(fc1_w, f)[1],
        G_E0: 2.0 * g0,
        G_E1: 2.0 * g1,
        G_Y0: hv0,
        G_Y1: hv1,
    }
    # R in the basis (u = cos(ang), w = sin(ang), 1); row 2T is the constant.
    R = np.zeros((KB, NG * T), dtype=f)
    dd = np.eye(T, dtype=f)
    for gi in range(NG):
        cols = slice(gi * T, (gi + 1) * T)
        if gi == G_A:
            R[0:T, cols] = M0.T
            R[T:2 * T, cols] = M1.T
            R[2 * T, cols] = c_t
        elif gi == G_R:
            b2 = (oB ** 2).sum()
            R[0:T, cols] = 2 * A * oB[0] * dd + b2 * M0.T
            R[T:2 * T, cols] = 2 * A * oB[1] * dd + b2 * M1.T
            R[2 * T, cols] = 2 * (pos * oB[None, 2:]).sum(-1) + b2 * c_t
        else:
            v = projs[gi]
            bv = (oB * v).sum()
            R[0:T, cols] = A * v[0] * dd + bv * M0.T
            R[T:2 * T, cols] = A * v[1] * dd + bv * M1.T
            R[2 * T, cols] = (pos * v[None, 2:]).sum(-1) + bv * c_t

    # fold 1/D into the e-groups and r-group
    R[:, G_E0 * T:(G_E1 + 1) * T] *= 1.0 / D
    R[:, G_R * T:(G_R + 1) * T] *= 1.0 / D

    G00, G01, G11 = (g0 * g0).sum(), (g0 * g1).sum(), (g1 * g1).sum()
    if G00 > 1e-30:
        sq0, rat = np.sqrt(G00), G01 / G00
        c3 = np.sqrt(max(G11 - G01 * G01 / G00, 0.0))
    else:
        sq0, rat, c3 = 0.0, 0.0, np.sqrt(G11)
    sc05 = np.sqrt(1.0 / D)

    H = np.array([[(g0 * hv0).sum(), (g0 * hv1).sum()],
                  [(g1 * hv0).sum(), (g1 * hv1).sum()]])
    dvoc = np.arange(V, dtype=f)
    ang = f(tok_start) + dvoc * f(tok_stride)
    E = np.stack([A * np.cos(ang), A * np.sin(ang)], axis=-1)  # (V, 2)
    # per-quantity v-rows for the normalized part
    QR = np.stack([H[0, 0] * E[:, 0] + H[0, 1] * E[:, 1],
                   H[1, 0] * E[:, 0] + H[1, 1] * E[:, 1],
                   E[:, 0], E[:, 1]], axis=0)  # (4, V)

    # RHS2 [KB, N2]: normalized rows q*TM + t (t < TM) at cols t*V+v;
    # l-rows LOFF + c*TL + dt (t = TM+dt) with E-columns
    RHS2 = np.zeros((KB, N2), dtype=f)
    for qq in range(4):
        for tt in range(TM):
            RHS2[qq * TM + tt, tt * V:(tt + 1) * V] = QR[qq]
    for cc in range(2):
        for dt_ in range(TL):
            t_ = TM + dt_
            RHS2[LOFF + cc * TL + dt_, t_ * V:(t_ + 1) * V] = E[:, cc]

    # S' = S/D + EPS, tiled per supergroup; shipped as f16
    SROW = np.tile(S_t / D + EPS, SGB)[None, :]

    PRM = np.zeros((1, NPRM), dtype=np.float32)
    PRM[0, P_SSC] = f(tok_stride)
    PRM[0, P_SB] = f(tok_start)
    PRM[0, P_CSC] = -f(tok_stride)
    PRM[0, P_CB] = np.pi / 2.0 - f(tok_start)
    PRM[0, P_ZERO] = 0.0
    sc = {"rat": float(rat), "sq0": float(sq0 * sc05), "c3": float(c3 * sc05),
          "h00": float(H[0, 0]), "h10": float(H[1, 0]),
          "h01": float(H[0, 1]), "h11": float(H[1, 1])}
    return (np.ascontiguousarray(R.astype(F16NP)),
            np.ascontiguousarray(RHS2.astype(F16NP)),
            np.ascontiguousarray(SROW.astype(F16NP)),
            PRM, sc)


def _act_rsqrt(nc, out, in_):
    """ACT Rsqrt via direct InstActivation (wrapper bans it for accuracy;
    fine at this kernel's 2e-2 tolerance)."""
    eng = nc.scalar
    inputs = [eng.lower_ap(in_)]
    for arg in (0.0, 1.0, 0.0):  # bias, scale, alpha
        inputs.append(mybir.ImmediateValue(dtype=mybir.dt.float32, value=arg))
    return eng.add_instruction(
        mybir.InstActivation(
            name=eng.bass.get_next_instruction_name(),
            func=AF.Rsqrt,
            ins=inputs,
            outs=[eng.lower_ap(out)],
        )
    )


# ---------------------------------------------------------------- bass kernel
def build_bass(sc=None):
    """sc: chain scalars baked as compile-time immediates (AP-scalar
    TensorScalar ops hit a ~7us/op slow path on HW)."""
    if sc is None:
        sc = {"rat": 0.0, "sq0": 1.0, "c3": 1.0,
              "h00": 0.0, "h10": 0.0, "h01": 0.0, "h11": 0.0}
    nc = bacc.Bacc("TRN2", target_bir_lowering=False, debug=False)

    idx_d = nc.dram_tensor("idx", [BC, T], U8, kind="ExternalInput").ap()
    r_d = nc.dram_tensor("R", [KB, N1], F16, kind="ExternalInput").ap()
    rhs2_d = nc.dram_tensor("RHS2", [KB, N2], F16, kind="ExternalInput").ap()
    srow_d = nc.dram_tensor("SROW", [1, FW], F16, kind="ExternalInput").ap()
    prm_d = nc.dram_tensor("PRM", [1, NPRM], F32, kind="ExternalInput").ap()
    out_d = nc.dram_tensor("out", [BC, N2], F16, kind="ExternalOutput").ap()

    out_v8 = out_d.rearrange("(p g f) c -> p g (f c)", p=P, f=8)  # [128,8,3808]

    with tile.TileContext(nc) as tc:
        with (
            tc.tile_pool(name="const", bufs=1) as cpool,
            tc.tile_pool(name="uwp", bufs=1) as uwpool,
            tc.tile_pool(name="uwt", bufs=8) as uwtp,
            tc.tile_pool(name="dr", bufs=2) as drp,
            tc.tile_pool(name="sg", bufs=2) as sgp,
            tc.tile_pool(name="lit", bufs=2) as litp,
            tc.tile_pool(name="outsb", bufs=3) as outp,
            tc.tile_pool(name="pmm1", bufs=2, space="PSUM") as pmm1p,
            tc.tile_pool(name="pout", bufs=2, space="PSUM") as poutp,
        ):
            # ---- inputs: prm + idx first (they gate the Sins), all on the
            # sync queue so the ACT queue is free to run phase A immediately
            prm_sb = cpool.tile([P, NPRM], F32)
            nc.sync.dma_start(prm_sb[:], prm_d.broadcast_to([P, NPRM]))
            idx_t = cpool.tile([P, NBLK * T], U8)
            nc.sync.dma_start(idx_t[:], idx_d.rearrange("(p n) t -> p (n t)", p=P))
            r_sb = cpool.tile([KB, N1], F16)
            nc.sync.dma_start(r_sb[:], r_d)
            rhs2_sb = cpool.tile([KB, N2], F16)
            nc.sync.dma_start(rhs2_sb[:], rhs2_d)
            s_sb = cpool.tile([P, FW], F16)
            nc.sync.dma_start(s_sb[:], srow_d.broadcast_to([P, FW]))

            def prm(i):
                return prm_sb[:, i:i + 1]

            # dummy Sin pulls the trig ACT_TABLE_LOAD to t=0 (no input deps)
            dummy = cpool.tile([P, 1], F16)
            one_ap = nc.const_aps.tensor(1.0, (P, 1))
            nc.scalar.activation(dummy[:], one_ap, AF.Sin, bias=0.0, scale=0.1)

            # ---------------- phase A: idx -> basis [cos, sin, 1] (f16);
            # ACT Sin reads the u8 ids directly (args within [-pi, pi])
            uw = uwpool.tile([P, NBLK, KB], F16)
            idx3 = idx_t[:].rearrange("p (n t) -> p n t", t=T)
            nc.vector.memset(uw[:, :, 2 * T:2 * T + 1], 1.0)
            nc.gpsimd.memset(uw[:, :, 2 * T + 1:KB], 0.0)

            def phase_a(c):
                blk = slice(c * SGB, (c + 1) * SGB)
                nc.scalar.activation(uw[:, blk, 0:T], idx3[:, blk, :], AF.Sin,
                                     bias=prm(P_CB), scale=prm(P_CSC))
                nc.scalar.activation(uw[:, blk, T:2 * T], idx3[:, blk, :], AF.Sin,
                                     bias=prm(P_SB), scale=prm(P_SSC))

            def t1_transpose(c):
                j0 = c * 8
                uwT = uwtp.tile([KB, 8 * P], F16, tag="uwT")
                uwT3 = uwT[:].rearrange("k (j m) -> k j m", m=P)
                nc.sync.dma_start(
                    uwT3, uw[:, j0:j0 + 8, :].rearrange("p j k -> p (j k)"),
                    transpose=True)
                return uwT3

            def mm1_drains(sg, uwT3s):
                """Per h (2 blocks): 2 matmuls, one ACT Relu (q groups), one
                copy (non-q groups, f16; engine alternates)."""
                rho = drp.tile([P, SGB, 2 * T], F16, tag="rho")
                nonq = drp.tile([P, SGB, NQW], F16, tag="nonq")
                for h in range(SGB // 2):
                    jj = 2 * h
                    uwT3 = uwT3s[2 * sg + (1 if jj >= 8 else 0)]
                    pm = pmm1p.tile([P, 2, 512], F32, tag="pm")
                    for b in range(2):
                        nc.tensor.matmul(
                            pm[:, b, 0:N1],
                            uwT3[0:2 * T + 1, (jj % 8) + b, :],
                            r_sb[0:2 * T + 1, :],
                            start=True, stop=True)
                    sl = slice(jj, jj + 2)
                    nc.scalar.activation(rho[:, sl, :], pm[:, :, 0:2 * T],
                                         AF.Relu, bias=0.0, scale=1.0)
                    if h % 2 == 0:
                        nc.vector.tensor_copy(nonq[:, sl, :],
                                              pm[:, :, 2 * T:NG * T])
                    else:
                        nc.scalar.copy(nonq[:, sl, :], pm[:, :, 2 * T:NG * T])
                return rho, nonq

            def nq(t, g):
                return t[:, :, g * T:(g + 1) * T]

            def chain(sg, dr, lc):
                """f16 elementwise chain -> lint [P, 16, 128]. Emitted FIRST
                in the iteration so its ACT/DVE ops head the engine queues."""
                rho, nonq = dr
                rho0 = rho[:, :, 0:T]
                rho1 = rho[:, :, T:2 * T]

                tab = sgp.tile([P, SGB, 2 * T], F16, tag="tab")
                nc.vector.tensor_mul(tab[:],
                                     nonq[:, :, NQ_E0 * T:(NQ_E1 + 1) * T], rho)
                ar = sgp.tile([P, FW], F16, tag="ar")
                nc.vector.tensor_mul(ar[:].rearrange("p (n t) -> p n t", t=T),
                                     nq(nonq, NQ_A), nq(nonq, NQ_R))
                m2 = sgp.tile([P, FW], F16, tag="m2")
                nc.vector.tensor_add(m2[:], ar[:], s_sb[:])
                inv2 = sgp.tile([P, SGB, 1, T], F16, tag="inv2")
                inv2f = inv2[:].rearrange("p n c t -> p (n c t)")
                _act_rsqrt(nc, inv2f, m2[:])

                tau = sgp.tile([P, FW], F16, tag="tau")
                nc.gpsimd.tensor_add(tau[:].rearrange("p (n t) -> p n t", t=T),
                                     tab[:, :, 0:T], tab[:, :, T:2 * T])

                z = sgp.tile([P, SGB, 2, T], F16, tag="z")
                iv2b = inv2[:].to_broadcast([P, SGB, 2, T])
                nc.vector.tensor_mul(z[:], rho[:].rearrange(
                    "p n (c t) -> p n c t", t=T), iv2b)
                z0f = z[:, :, 0, :]
                z1f = z[:, :, 1, :]
                it2 = sgp.tile([P, FW], F16, tag="it2")
                nc.vector.tensor_mul(it2[:], tau[:], inv2f)

                z1r = sgp.tile([P, FW], F16, tag="z1r")
                nc.vector.tensor_scalar_mul(
                    z1r[:].rearrange("p (n t) -> p n t", t=T), z1f, sc["rat"])
                v1 = sgp.tile([P, FW], F16, tag="v1")
                nc.vector.tensor_add(v1[:].rearrange("p (n t) -> p n t", t=T),
                                     z1r[:].rearrange("p (n t) -> p n t", t=T),
                                     z0f)
                v1sq = sgp.tile([P, FW], F16, tag="v1sq")
                nc.scalar.activation(v1sq[:], v1[:], AF.Square,
                                     bias=prm(P_ZERO), scale=sc["sq0"])
                v2sq = sgp.tile([P, FW], F16, tag="v2sq")
                nc.scalar.activation(v2sq[:].rearrange("p (n t) -> p n t", t=T),
                                     z1f, AF.Square,
                                     bias=prm(P_ZERO), scale=sc["c3"])

                m3a = sgp.tile([P, FW], F16, tag="m3a")
                nc.vector.tensor_add(m3a[:], m2[:], it2[:])
                m3b = sgp.tile([P, FW], F16, tag="m3b")
                nc.vector.tensor_add(m3b[:], v1sq[:], v2sq[:])
                m3 = sgp.tile([P, FW], F16, tag="m3")
                nc.vector.tensor_add(m3[:], m3a[:], m3b[:])
                inv3 = sgp.tile([P, SGB, 1, T], F16, tag="inv3")
                inv3f = inv3[:].rearrange("p n c t -> p (n c t)")
                _act_rsqrt(nc, inv3f, m3[:])
                inv33 = inv3[:, :, 0, :]

                # lint writes: 4 normalized rows for t<TM (pair-merged via
                # stride-0 inv3 broadcast)
                z03 = z[:, :, 0, :]
                z13 = z[:, :, 1, :]
                y03 = nq(nonq, NQ_Y0)
                y13 = nq(nonq, NQ_Y1)
                ivm = inv33[:, :, 0:TM]
                ivmb = inv3[:, :, :, 0:TM].to_broadcast([P, SGB, 2, TM])
                nc.vector.tensor_mul(
                    lc[:, :, 0:2 * TM].rearrange("p n (c t) -> p n c t", t=TM),
                    z[:, :, :, 0:TM], ivmb)
                nc.gpsimd.tensor_mul(
                    lc[:, :, 2 * TM:4 * TM].rearrange(
                        "p n (c t) -> p n c t", t=TM),
                    nonq[:, :, NQ_Y0 * T:(NQ_Y1 + 1) * T].rearrange(
                        "p n (c t) -> p n c t", t=T)[:, :, :, 0:TM], ivmb)
                # ... and l-rows for the 4 leftover t's (classic p-chain on
                # [P, 16, 4] slices)
                tl = slice(TM, T)
                zl0 = z03[:, :, tl]
                zl1 = z13[:, :, tl]
                _ = (zl0, zl1)
                ivl = inv33[:, :, tl]
                ta = sgp.tile([P, SGB, TL], F16, tag="ta")
                nc.vector.scalar_tensor_tensor(ta[:], zl1, sc["h10"],
                                               y03[:, :, tl],
                                               op0=ALU.mult, op1=ALU.add)
                tb = sgp.tile([P, SGB, TL], F16, tag="tb")
                nc.vector.scalar_tensor_tensor(tb[:], zl0, sc["h00"], ta[:],
                                               op0=ALU.mult, op1=ALU.add)
                nc.gpsimd.tensor_mul(lc[:, :, LOFF:LOFF + TL], tb[:], ivl)
                tcn = sgp.tile([P, SGB, TL], F16, tag="tc")
                nc.vector.scalar_tensor_tensor(tcn[:], zl1, sc["h11"],
                                               y13[:, :, tl],
                                               op0=ALU.mult, op1=ALU.add)
                td = sgp.tile([P, SGB, TL], F16, tag="td")
                nc.vector.scalar_tensor_tensor(td[:], zl0, sc["h01"], tcn[:],
                                               op0=ALU.mult, op1=ALU.add)
                nc.gpsimd.tensor_mul(lc[:, :, LOFF + TL:KB], td[:], ivl)

            def t2_transpose(lc, lcT3):
                nc.sync.dma_start(
                    lcT3, lc[:].rearrange("p j k -> p (j k)"), transpose=True)

            def mm2_copies(sg, lcT3):
                stores = []
                for half in range(2):
                    o_sb = outp.tile([P, 8, N2], F16, tag="osb")
                    for hh in range(4):
                        jj = 8 * half + 2 * hh
                        po = poutp.tile([P, 2, 512], F32, tag="po")
                        for b in range(2):
                            nc.tensor.matmul(po[:, b, 0:N2],
                                             lcT3[0:P, jj + b, :],
                                             rhs2_sb[0:P, :],
                                             start=True, stop=True)
                        if hh % 2 == 0:
                            nc.scalar.copy(o_sb[:, 2 * hh:2 * hh + 2, :],
                                           po[:, :, 0:N2])
                        else:
                            nc.vector.tensor_copy(o_sb[:, 2 * hh:2 * hh + 2, :],
                                                  po[:, :, 0:N2])
                    stores.append((sg * 2 + half, o_sb))
                return stores

            def emit_stores(stores):
                for gi, o_sb in stores:
                    nc.sync.dma_start(out_v8[:, gi, :],
                                      o_sb[:].rearrange("p f c -> p (f c)"))

            # persistent lint tiles (fully written each sg -> no pad memset)
            lints = []
            for i in range(2):
                lc = uwpool.tile([P, SGB, KB], F16, tag=f"lint{i}")
                lcT = litp.tile([KB, SGB * P], F16, tag="lintT")
                lints.append((lc, lcT[:].rearrange("k (j m) -> k j m", m=P)))

            # ---------------- driver
            uwTs = []
            for sg in range(NSG):
                phase_a(sg)
                uwTs.append(t1_transpose(2 * sg))
                uwTs.append(t1_transpose(2 * sg + 1))

            dr = mm1_drains(0, uwTs)
            pend = []
            for sg in range(NSG):
                lc, lcT3 = lints[sg % 2]
                chain(sg, dr, lc)
                t2_transpose(lc, lcT3)
                emit_stores(pend)
                dr = mm1_drains(sg + 1, uwTs) if sg + 1 < NSG else None
                pend = mm2_copies(sg, lcT3)
            emit_stores(pend)

    nc.compile()
    return nc


_CACHE = {}


def _get_nc(sc):
    key = tuple(sorted(sc.items()))
    if _CACHE.get("key") != key:
        _CACHE["nc"] = build_bass(sc)
        _CACHE["key"] = key
    return _CACHE["nc"]


def kernel(**inputs) -> np.ndarray:
    idx = np.asarray(inputs["idx"]).astype(np.uint8)
    kw = {k: np.asarray(v, np.float64) for k, v in inputs.items() if k != "idx"}
    R, RHS2, SROW, PRM, sc = host_tables(**kw)
    nc = _get_nc(sc)
    in_maps = [
        {"idx": idx[c * BC:(c + 1) * BC], "R": R, "RHS2": RHS2,
         "SROW": SROW, "PRM": PRM}
        for c in range(NCORES)
    ]
    res = run_bass_kernel_spmd(nc, in_maps, core_ids=list(range(NCORES)))
    out = np.concatenate([res.results[c]["out"] for c in range(NCORES)], axis=0)
    return np.ascontiguousarray(out.astype(np.float32).reshape(B, T, V))
